# revision 38
# baseline (speedup 1.0000x reference)
"""AdaptiveTripletLoss on 8 Trainium2 NeuronCores (Bass/Tile).

Strategy
--------
Rows (samples) are sorted by class label and sharded 512/core.  Every core
gets the full bf16-transposed feature matrix with its *columns rolled* so
that the core's own rows sit at local columns [128, 640); since columns are
class-sorted, every row's same-class window then lives inside local column
tiles {0, 1} -- identical geometry on all 8 cores (SPMD-uniform graph).

Per core the TensorEngine computes h = G - s_i/2 - s_j/2 = -d2/2 directly
in PSUM via an augmented K=4 bf16 matmul (hi/lo split of -||f||^2/2 rows).
ScalarE produces dist = sqrt(-2h + eps) (float32r).  VectorE computes
  * hardest_neg  via an index-window *exclusion* masked max over h
  * hardest_pos  via an index-window inclusion masked max over dist
  * sum_pos      via a custom MASK_SUM windowed-sum op
and TensorE folds per-column dist sums (ones-vector f32r matmuls); by
symmetry of the distance matrix the full row sums are recovered host-side
from the 8 partial column sums.  Class-mean statistics are label-derived
prep computed on the host; the device does the O(N*C*D) feats @ cmean^T
matmul plus one-hot gathers and finishes stat_margin per row.  The final
O(N) scalar reduction runs on the host from the per-row outputs.
"""

import numpy as np

N = 4096
D = 512
NCLS = 64
NCORES = 8
RPC = N // NCORES          # rows per core
RB = RPC // 128            # row blocks per core (4)
TT = 512                   # column tile width
NT = N // TT               # column tiles (8)
WT = 2                     # window tiles (local tiles 0,1)
EPS = 0.05                 # d2 shift so sqrt never sees negatives
BASE_MARGIN = 0.1
ADAPTIVE_WEIGHT = 0.1
STAT_WEIGHT = 0.1

_BUILT = None
LAST_EXEC_NS = None
LAST_TRACE_DIR = None


def _maybe_enable_trace():
    """If BASS_KERNEL_TRACE=1, install the antenv.axon_hooks shim so
    run_bass_kernel_spmd(trace=True) can capture an NTFF profile under axon."""
    import os
    if os.environ.get("BASS_KERNEL_TRACE") != "1":
        return False
    import sys as _sys
    import types
    if "antenv.axon_hooks" not in _sys.modules:
        mod = types.ModuleType("antenv.axon_hooks")
        mod._hook = None
        mod.set_axon_ntff_profile_hook = lambda h: setattr(mod, "_hook", h)
        mod.get_axon_ntff_profile_hook = lambda: mod._hook
        _sys.modules["antenv.axon_hooks"] = mod
        try:
            from trn_agent_boot.trn_boot import _ntff_profile_via_ctypes
            mod._hook = _ntff_profile_via_ctypes("/opt/axon/libaxon_pjrt.so")
        except Exception:
            return False
    return _sys.modules["antenv.axon_hooks"]._hook is not None


def _register_mask_sum():
    """Author the MASK_SUM custom DVE op (windowed sum with TMR-style
    wrap/invert index mask; sentinel 0 instead of -FLT_MAX)."""
    from concourse import dve_ops
    from concourse.dve_ops import DveOp, OPS, _SUB_OPCODE_FOR_NAME, _CUSTOM_DVE_ROW_BASE
    from concourse.dve_spec import (
        C0, C1, C2, C3, Idx, Spec, Zero, _spill_c3_to_src1, lower, maxx, minn, select,
    )
    from concourse.dve_uop import DveOpSpec
    from operator import add

    name = "MASK_SUM_ANT"
    if name in _SUB_OPCODE_FOR_NAME:
        return next(op for op in OPS if op.name == name)

    def _ref(in0, in1, s0, s1, imm2):
        P = in0.shape[0]
        x = in0.reshape(P, -1).astype(np.float32)
        n = x.shape[1]
        start = np.broadcast_to(np.asarray(s0, np.float32).reshape(-1, 1), (P, 1))
        end = np.broadcast_to(np.asarray(in1, np.float32).reshape(-1, 1), (P, 1))
        idx = np.arange(n, dtype=np.float32)[None, :]
        mask = (idx >= np.minimum(start, end)) & (idx < np.maximum(start, end))
        mask = np.where(start > end, ~mask, mask)
        body = np.where(mask, x, 0.0) * np.float32(imm2)
        acc = np.asarray(s1, np.float32).reshape(-1, 1) + body.sum(1, keepdims=True)
        return body.reshape(in0.shape), acc.astype(np.float32)

    _mask_idx = ((Idx >= minn(C0, C3)) & (Idx < maxx(C0, C3))) ^ (C0 > C3)
    body = _spill_c3_to_src1(select(_mask_idx, dve_ops.Src0, Zero) * C2)
    spec = Spec(body=body, accum=add, accum_init=C1, reference=_ref)
    shas = {}
    for ver in ("v3", "v4"):
        try:
            shas[ver] = DveOpSpec(name=name, opcode=0, uops=lower(spec, ver=ver),
                                  rd1_en=True).sha(ver)
        except Exception:
            pass
    op = DveOp(name, spec, subdim=False, uops_sha=shas)
    OPS.append(op)
    _SUB_OPCODE_FOR_NAME[name] = _CUSTOM_DVE_ROW_BASE + len(OPS) - 1
    dve_ops.CUSTOM_DVE_SPECS[name] = spec
    return op


def _build():
    """Compile the SPMD Bass graph (once per process)."""
    global _BUILT
    if _BUILT is not None:
        return _BUILT

    import concourse.bacc as bacc
    import concourse.mybir as mybir
    from concourse import tile
    from concourse import dve_ops

    MASK_SUM = _register_mask_sum()
    TMR = dve_ops.TENSOR_MASK_REDUCE

    f32 = mybir.dt.float32
    f32r = mybir.dt.float32r
    bf16 = mybir.dt.bfloat16

    nc = bacc.Bacc("TRN2", target_bir_lowering=False, debug=False,
                   num_devices=NCORES)

    # ---- DRAM I/O -------------------------------------------------------
    d_ftT = nc.dram_tensor("ftT", [D, N], bf16, kind="ExternalInput").ap()
    d_auglhs = nc.dram_tensor("auglhs", [4, RPC], bf16, kind="ExternalInput").ap()
    d_augrhs = nc.dram_tensor("augrhs", [4, N], bf16, kind="ExternalInput").ap()
    # cmean^T k-tiles [128, 64] packed side by side: cmT[:, 64k:64k+64]
    d_cmT = nc.dram_tensor("cmT", [128, 4 * NCLS], bf16, kind="ExternalInput").ap()
    d_gb = nc.dram_tensor("gb", [NCLS, 2], f32r, kind="ExternalInput").ap()
    d_oh = nc.dram_tensor("oh", [RPC, NCLS], f32, kind="ExternalInput").ap()
    d_ohT = nc.dram_tensor("ohT", [NCLS, RPC], f32r, kind="ExternalInput").ap()
    d_rc = nc.dram_tensor("rc", [RPC, 32], f32, kind="ExternalInput").ap()
    d_ones = nc.dram_tensor("onesr", [128, 1], f32r, kind="ExternalInput").ap()
    o_rows = nc.dram_tensor("o_rows", [RPC, 8], f32, kind="ExternalOutput").ap()
    o_cs = nc.dram_tensor("o_cs", [1, N], f32, kind="ExternalOutput").ap()

    with tile.TileContext(nc) as tc:
        with (
            tc.tile_pool(name="const", bufs=1) as cp,
            tc.tile_pool(name="dist", bufs=14) as dp,
            tc.tile_pool(name="scr", bufs=4) as sp,
            tc.tile_pool(name="acc", bufs=2) as ap_,
            tc.tile_pool(name="fin", bufs=1) as fp,
            tc.tile_pool(name="psh", bufs=6, space="PSUM") as psh,
            tc.tile_pool(name="pss", bufs=1, space="PSUM") as pss,
            tc.tile_pool(name="psc", bufs=1, space="PSUM") as psc,
        ):
            # ---- load constants -----------------------------------------
            # ftT: 4 large DMAs (one per k-tile) so the HW DGE fans each out
            # across many queues; small constants ride other sequencers.
            ft = [cp.tile([128, N], bf16, tag=f"ft{k}", name=f"ft{k}") for k in range(4)]
            # chunk A (local cols 0..1023: own rows + window region) lands
            # first so the main loop starts early; B and C follow.
            for k in range(4):
                nc.sync.dma_start(ft[k][:, 0:640], d_ftT[k * 128:(k + 1) * 128, 0:640])
            for k in range(4):
                nc.scalar.dma_start(ft[k][:, 640:2048],
                                    d_ftT[k * 128:(k + 1) * 128, 640:2048])
            for k in range(4):
                nc.sync.dma_start(ft[k][:, 2048:N],
                                  d_ftT[k * 128:(k + 1) * 128, 2048:N])
            auglhs = cp.tile([4, RPC], bf16)
            augrhs = cp.tile([4, N], bf16)
            nc.gpsimd.dma_start(auglhs[:], d_auglhs[:])
            nc.gpsimd.dma_start(augrhs[:], d_augrhs[:])
            oh = [cp.tile([128, NCLS], f32, tag=f"oh{k}", name=f"oh{k}") for k in range(4)]
            for k in range(4):
                nc.scalar.dma_start(oh[k][:], d_oh[k * 128:(k + 1) * 128, :])
            ohT = cp.tile([NCLS, RPC], f32r)
            nc.scalar.dma_start(ohT[:], d_ohT[:])
            # per-row constants, one [128, 32] tile per rowblock:
            # [0:8]=mns [8:16]=mne [16:18]=wps [18:20]=wpe [20]=sown
            rc = [cp.tile([128, 32], f32, tag=f"rc{r}", name=f"rc{r}") for r in range(RB)]
            for r in range(RB):
                nc.gpsimd.dma_start(rc[r][:], d_rc[r * 128:(r + 1) * 128, :])
            onesr = cp.tile([128, 1], f32r)
            nc.gpsimd.dma_start(onesr[:], d_ones[:])
            cmTg = cp.tile([128, 4 * NCLS], bf16)
            nc.sync.dma_start(cmTg[:], d_cmT[:])
            cmT = [cmTg[:, k * NCLS:(k + 1) * NCLS] for k in range(4)]
            gb = cp.tile([NCLS, 2], f32r)
            nc.sync.dma_start(gb[:], d_gb[:])
            mns = [rc[r][:, 0:NT] for r in range(RB)]
            mne = [rc[r][:, NT:2 * NT] for r in range(RB)]
            wps = [rc[r][:, 16:17] for r in range(RB)]
            wpe = [rc[r][:, 17:18] for r in range(RB)]
            sown = [rc[r][:, 20:21] for r in range(RB)]
            epst = cp.tile([128, 1], f32)
            nc.vector.memset(epst[:], EPS)

            orows_sb = [fp.tile([128, 8], f32, tag=f"or{r}", name=f"orows{r}") for r in range(RB)]
            for r in range(RB):
                nc.vector.memset(orows_sb[r][:], 0.0)
            cs_sb = fp.tile([1, N], f32)

            # ================= Phase A: class stats ======================
            # cmean / cm2 / uncertainty are label-stat prep computed on the
            # host; the device does the O(N*C*D) FCM matmul + gathers.
            # per-rowblock: FCM, gathers, stat_margin -> orows col 3
            def emit_stats(r):
                own = slice(128 + r * 128, 256 + r * 128)   # local cols of own rows
                fcm_ps = pss.tile([128, NCLS], f32, tag="stat")
                for k in range(4):
                    nc.tensor.matmul(fcm_ps[:], ft[k][:, own], cmT[k],
                                     start=(k == 0), stop=(k == 3))
                g_ps = pss.tile([128, 2], f32, tag="stat")
                nc.tensor.matmul(g_ps[:], ohT[:, r * 128:(r + 1) * 128], gb[:],
                                 start=True, stop=True)
                scr64 = sp.tile([128, NCLS], f32, tag="scr64")
                nc.vector.scalar_tensor_tensor(
                    out=scr64[:], in0=oh[r][:], scalar=1.0, in1=fcm_ps[:],
                    op0=mybir.AluOpType.mult, op1=mybir.AluOpType.mult)
                fcm_g = ap_.tile([128, 1], f32, tag=f"fg{r}")
                nc.vector.tensor_reduce(fcm_g[:], scr64[:], axis=mybir.AxisListType.X,
                                        op=mybir.AluOpType.add)
                cd2 = ap_.tile([128, 1], f32, tag=f"cd{r}")
                # cd2 = fcm_g * (-2) + cm2_g
                nc.vector.scalar_tensor_tensor(
                    out=cd2[:], in0=fcm_g[:], scalar=-2.0, in1=g_ps[:, 1:2],
                    op0=mybir.AluOpType.mult, op1=mybir.AluOpType.add)
                nc.vector.tensor_tensor(cd2[:], cd2[:], sown[r][:],
                                        op=mybir.AluOpType.add)
                nc.vector.tensor_scalar_max(cd2[:], cd2[:], 0.0)
                cd = ap_.tile([128, 1], f32, tag=f"cdq{r}")
                nc.scalar.activation(cd[:], cd2[:], mybir.ActivationFunctionType.Sqrt)
                nc.vector.tensor_tensor(orows_sb[r][:, 3:4], cd[:], g_ps[:, 0:1],
                                        op=mybir.AluOpType.mult)

            # ================= Phase B: distance matrix ==================
            maxh = [[None] * (NT + 1) for _ in range(RB)]
            dists = [[None] * RB for _ in range(NT)]
            wdist = [sp.tile([128, 2 * TT], f32r, tag=f"wd{r}", name=f"wd{r}",
                             bufs=1) for r in range(RB)]

            def emit_colsum(t):
                csum_ps = psc.tile([1, TT], f32, tag="cs", name=f"cs{t}")
                for r in range(RB):
                    nc.tensor.matmul(csum_ps[:], onesr[:], dists[t][r],
                                     start=(r == 0), stop=(r == RB - 1))
                nc.scalar.copy(cs_sb[:, t * TT:(t + 1) * TT], csum_ps[:])

            for t in range(NT):
                for r in range(RB):
                    own = slice(128 + r * 128, 256 + r * 128)
                    cols = slice(t * TT, (t + 1) * TT)
                    h_ps = psh.tile([128, TT], f32, tag="h")
                    for k in range(4):
                        nc.tensor.matmul(h_ps[:], ft[k][:, own], ft[k][:, cols],
                                         start=(k == 0), stop=False)
                    nc.tensor.matmul(h_ps[:], auglhs[:, r * 128:(r + 1) * 128],
                                     augrhs[:, cols], start=False, stop=True)
                    # dist = sqrt(-2h + eps)   (f32r output); the two window
                    # tiles (t<2) land side by side in wdist[r]
                    if t < WT:
                        dt_ = wdist[r][:, t * TT:(t + 1) * TT]
                    else:
                        dt_ = dp.tile([128, TT], f32r, tag="dist", name="dtile")[:]
                    nc.scalar.activation(dt_, h_ps[:],
                                         mybir.ActivationFunctionType.Sqrt,
                                         bias=epst[:], scale=-2.0)
                    # hneg: exclusion masked max of h, chained over t
                    scr = sp.tile([128, TT], f32, tag="scr")
                    prev = maxh[r][t] if t > 0 else -1e30
                    if t == NT - 1:
                        acc_mh = orows_sb[r][:, 1:2]
                    else:
                        nxt = ap_.tile([128, 1], f32, tag=f"mh{r}")
                        maxh[r][t + 1] = nxt
                        acc_mh = nxt[:]
                    nc.vector._custom_dve(
                        TMR, out=scr[:], in0=h_ps[:], in1=mne[r][:, t:t + 1],
                        s0=mns[r][:, t:t + 1],
                        s1=(prev[:] if t > 0 else prev), imm2=1.0,
                        accum_out=acc_mh)
                    if t == WT - 1:
                        # windowed hpos / sum_pos over ONE 512-wide slice of
                        # wdist at column offset 128r (covers the whole class
                        # window of this rowblock's rows)
                        wsl = wdist[r][:, 128 * r:128 * r + TT].bitcast(f32)
                        scr2 = sp.tile([128, TT], f32, tag="scr")
                        nc.vector._custom_dve(
                            TMR, out=scr2[:], in0=wsl, in1=wpe[r],
                            s0=wps[r], s1=-1e30, imm2=1.0,
                            accum_out=orows_sb[r][:, 0:1])
                        scr3 = sp.tile([128, TT], f32, tag="scr")
                        nc.vector._custom_dve(
                            MASK_SUM, out=scr3[:], in0=wsl, in1=wpe[r],
                            s0=wps[r], s1=0.0, imm2=1.0,
                            accum_out=orows_sb[r][:, 2:3])
                    dists[t][r] = dt_
                # column sums (partial row sums by symmetry), one tile behind
                # so the in-order PE stream never waits on ScalarE's sqrt
                if t > 0:
                    emit_colsum(t - 1)
                if t == 2:
                    for rr in range(RB):
                        emit_stats(rr)
            emit_colsum(NT - 1)

            # ---- outputs ------------------------------------------------
            for r in range(RB):
                nc.sync.dma_start(o_rows[r * 128:(r + 1) * 128, :], orows_sb[r][:])
            nc.sync.dma_start(o_cs[:], cs_sb[:])

    nc.compile()
    _BUILT = nc
    return nc


def _split_bf16(x32, mldt):
    hi = x32.astype(mldt.bfloat16)
    lo = (x32 - hi.astype(np.float32)).astype(mldt.bfloat16)
    return hi, lo


def kernel(feats, labels):
    import sys
    if "/opt/trn_rl_repo" not in sys.path:
        sys.path.insert(0, "/opt/trn_rl_repo")
    import ml_dtypes
    from concourse.bass_utils import run_bass_kernel_spmd

    feats_np = np.asarray(feats, dtype=np.float32)
    labels_np = np.asarray(labels)
    lab_i = labels_np.astype(np.int64)
    assert feats_np.shape == (N, D)

    # ---- host prep: sort by class --------------------------------------
    order = np.argsort(lab_i, kind="stable")
    ls = lab_i[order]
    fs = feats_np[order]
    cnt = np.bincount(ls, minlength=NCLS).astype(np.int64)
    seg_start = np.concatenate([[0], np.cumsum(cnt)[:-1]])
    ws_g = seg_start[ls].astype(np.int64)          # per sorted row: class col start
    we_g = (seg_start[ls] + cnt[ls]).astype(np.int64)

    fb = fs.astype(ml_dtypes.bfloat16)             # bf16 feats, sorted rows
    fb32 = fb.astype(np.float32)
    s_b = (fb32.astype(np.float64) ** 2).sum(1)    # ||bf16 f||^2 (f64)
    sh32 = (-(s_b / 2.0)).astype(np.float32)       # -s/2 in f32
    s_f32 = ((fs.astype(np.float64) ** 2).sum(1)).astype(np.float32)  # exact norms
    hi, lo = _split_bf16(sh32, ml_dtypes)

    ftT_g = np.ascontiguousarray(fb.T)             # [D, N] bf16, global col order

    onehot = np.zeros((N, NCLS), np.float32)
    onehot[np.arange(N), ls] = 1.0


    # class stats (host: label-derived prep at O(N*D))
    cnt_f = np.maximum(cnt, 1).astype(np.float64)
    cmean = np.zeros((NCLS, D), np.float64)
    np.add.at(cmean, ls, fs.astype(np.float64))
    cmean /= cnt_f[:, None]
    cmsq = np.zeros((NCLS, D), np.float64)
    np.add.at(cmsq, ls, fs.astype(np.float64) ** 2)
    cmsq /= cnt_f[:, None]
    cvar = np.maximum(cmsq - cmean ** 2, 0.0)        # per-dim clamp like the ref
    cm2 = (cmean ** 2).sum(1)                        # ||cmean_c||^2
    u = cvar.mean(1)                                 # mean_d cvar
    cmT_grp = np.ascontiguousarray(
        cmean.T.reshape(4, 128, NCLS).transpose(1, 0, 2).reshape(128, 4 * NCLS)
    ).astype(ml_dtypes.bfloat16)
    gb_host = np.stack([u, cm2], axis=1).astype(np.float32)

    in_maps = []
    for c in range(NCORES):
        roll = 512 * c - 128
        colperm = (np.arange(N) + roll) % N        # local j -> global col
        rows = slice(512 * c, 512 * (c + 1))
        lw = ws_g[rows] - roll                     # local window bounds per row
        le = we_g[rows] - roll
        assert lw.min() >= 0 and le.max() <= WT * TT, (lw.min(), le.max())

        rc_a = np.zeros((RPC, 32), np.float32)
        mns_a = rc_a[:, 0:NT]
        mne_a = rc_a[:, NT:2 * NT]
        rc_a[:, 20] = s_f32[rows]
        for t in range(NT):
            a = np.clip(lw - t * TT, 0, TT)
            b = np.clip(le - t * TT, 0, TT)
            inter = b > a
            # exclusion encoding for hneg: (start, end) = (b, a); else include-all
            mns_a[:, t] = np.where(inter, b, 0.0)
            mne_a[:, t] = np.where(inter, a, float(TT))
        # window bounds relative to the rowblock's 512-wide slice at 128*rb
        rb_of = np.arange(RPC) // 128
        rel_s = lw - 128 * rb_of
        rel_e = le - 128 * rb_of
        assert rel_s.min() >= 0 and rel_e.max() <= TT, (rel_s.min(), rel_e.max())
        rc_a[:, 16] = rel_s
        rc_a[:, 17] = rel_e

        augrhs = np.zeros((4, N), ml_dtypes.bfloat16)
        augrhs[0, :] = ml_dtypes.bfloat16(1.0)
        augrhs[1, :] = ml_dtypes.bfloat16(1.0)
        augrhs[2, :] = hi[colperm]
        augrhs[3, :] = lo[colperm]
        auglhs = np.zeros((4, RPC), ml_dtypes.bfloat16)
        auglhs[0, :] = hi[rows]
        auglhs[1, :] = lo[rows]
        auglhs[2, :] = ml_dtypes.bfloat16(1.0)
        auglhs[3, :] = ml_dtypes.bfloat16(1.0)

        in_maps.append({
            "ftT": np.ascontiguousarray(ftT_g[:, colperm]),
            "auglhs": auglhs,
            "augrhs": augrhs,
            "cmT": cmT_grp,
            "gb": gb_host,
            "oh": onehot[rows],
            "ohT": np.ascontiguousarray(onehot[rows].T.astype(np.float32)),
            "rc": rc_a,
            "onesr": np.ones((128, 1), np.float32),
        })

    nc = _build()
    trace = _maybe_enable_trace()
    import tempfile
    tmpdir = tempfile.mkdtemp(prefix="triplet_trace_") if trace else None
    res = run_bass_kernel_spmd(nc, in_maps, core_ids=list(range(NCORES)),
                               trace=bool(trace), tmpdir=tmpdir)
    global LAST_EXEC_NS, LAST_TRACE_DIR
    LAST_EXEC_NS = res.exec_time_ns
    LAST_TRACE_DIR = tmpdir

    # ---- host epilogue (O(N) numpy) ------------------------------------
    hpos = np.concatenate([res.results[c]["o_rows"][:, 0] for c in range(NCORES)])
    maxh = np.concatenate([res.results[c]["o_rows"][:, 1] for c in range(NCORES)])
    possum = np.concatenate([res.results[c]["o_rows"][:, 2] for c in range(NCORES)])
    statm = np.concatenate([res.results[c]["o_rows"][:, 3] for c in range(NCORES)])

    rowsum = np.zeros(N, np.float64)
    for c in range(NCORES):
        roll = 512 * c - 128
        colperm = (np.arange(N) + roll) % N
        part = res.results[c]["o_cs"][0].astype(np.float64)
        np.add.at(rowsum, colperm, part)

    hneg = np.sqrt(np.maximum(-2.0 * maxh.astype(np.float64) + EPS, 0.0))
    diag_dist = np.sqrt(EPS)
    pos_cnt = (cnt[ls] - 1).astype(np.float64)
    neg_cnt = (N - cnt[ls]).astype(np.float64)
    mean_pos = (possum.astype(np.float64) - diag_dist) / np.maximum(pos_cnt, 1.0)
    sum_neg = rowsum - possum.astype(np.float64)
    mean_neg = sum_neg / np.maximum(neg_cnt, 1.0)
    final_margin = (BASE_MARGIN + ADAPTIVE_WEIGHT * (mean_neg - mean_pos)
                    + STAT_WEIGHT * statm.astype(np.float64))
    per_sample = np.maximum(hpos.astype(np.float64) - hneg + final_margin, 0.0)
    valid = (pos_cnt > 0) & (neg_cnt > 0)
    n_valid = valid.sum()
    loss = per_sample[valid].sum() / max(n_valid, 1) if n_valid > 0 else 0.0
    return np.array(loss, dtype=np.float32)


if __name__ == "__main__":
    import jax
    key = jax.random.key(0)
    k1, k2 = jax.random.split(key)
    feats = np.asarray(jax.random.normal(k1, (N, D), dtype=np.float32))
    labels = np.asarray(jax.random.randint(k2, (N,), 0, NCLS, dtype=np.int32))
    out = kernel(feats=feats, labels=labels)
    print("kernel loss:", out)


# revision 39
# speedup vs baseline: 1.2082x; 1.2082x over previous
"""AdaptiveTripletLoss on 8 Trainium2 NeuronCores (Bass/Tile).

Strategy
--------
Rows (samples) are sorted by class label and sharded 512/core.  Every core
gets the full bf16-transposed feature matrix with its *columns rolled* so
that the core's own rows sit at local columns [128, 640); since columns are
class-sorted, every row's same-class window then lives inside local column
tiles {0, 1} -- identical geometry on all 8 cores (SPMD-uniform graph).

Per core the TensorEngine computes h = G - s_i/2 - s_j/2 = -d2/2 directly
in PSUM via an augmented K=4 bf16 matmul (hi/lo split of -||f||^2/2 rows).
ScalarE produces dist = sqrt(-2h + eps) (float32r).  VectorE computes
  * hardest_neg  via an index-window *exclusion* masked max over h
  * hardest_pos  via an index-window inclusion masked max over dist
  * sum_pos      via a custom MASK_SUM windowed-sum op
and TensorE folds per-column dist sums (ones-vector f32r matmuls); by
symmetry of the distance matrix the full row sums are recovered host-side
from the 8 partial column sums.  Class-mean statistics are label-derived
prep computed on the host; the device does the O(N*C*D) feats @ cmean^T
matmul plus one-hot gathers and finishes stat_margin per row.  The final
O(N) scalar reduction runs on the host from the per-row outputs.
"""

import numpy as np

N = 4096
D = 512
NCLS = 64
NCORES = 8
RPC = N // NCORES          # rows per core
RB = RPC // 128            # row blocks per core (4)
TT = 512                   # column tile width
NT = N // TT               # column tiles (8)
WT = 2                     # window tiles (local tiles 0,1)
EPS = 0.05                 # d2 shift so sqrt never sees negatives
BASE_MARGIN = 0.1
ADAPTIVE_WEIGHT = 0.1
STAT_WEIGHT = 0.1

_BUILT = None
LAST_EXEC_NS = None
LAST_TRACE_DIR = None


def _maybe_enable_trace():
    """If BASS_KERNEL_TRACE=1, install the antenv.axon_hooks shim so
    run_bass_kernel_spmd(trace=True) can capture an NTFF profile under axon."""
    import os
    if os.environ.get("BASS_KERNEL_TRACE") != "1":
        return False
    import sys as _sys
    import types
    if "antenv.axon_hooks" not in _sys.modules:
        mod = types.ModuleType("antenv.axon_hooks")
        mod._hook = None
        mod.set_axon_ntff_profile_hook = lambda h: setattr(mod, "_hook", h)
        mod.get_axon_ntff_profile_hook = lambda: mod._hook
        _sys.modules["antenv.axon_hooks"] = mod
        try:
            from trn_agent_boot.trn_boot import _ntff_profile_via_ctypes
            mod._hook = _ntff_profile_via_ctypes("/opt/axon/libaxon_pjrt.so")
        except Exception:
            return False
    return _sys.modules["antenv.axon_hooks"]._hook is not None


def _register_mask_sum():
    """Author the MASK_SUM custom DVE op (windowed sum with TMR-style
    wrap/invert index mask; sentinel 0 instead of -FLT_MAX)."""
    from concourse import dve_ops
    from concourse.dve_ops import DveOp, OPS, _SUB_OPCODE_FOR_NAME, _CUSTOM_DVE_ROW_BASE
    from concourse.dve_spec import (
        C0, C1, C2, C3, Idx, Spec, Zero, _spill_c3_to_src1, lower, maxx, minn, select,
    )
    from concourse.dve_uop import DveOpSpec
    from operator import add

    name = "MASK_SUM_ANT"
    if name in _SUB_OPCODE_FOR_NAME:
        return next(op for op in OPS if op.name == name)

    def _ref(in0, in1, s0, s1, imm2):
        P = in0.shape[0]
        x = in0.reshape(P, -1).astype(np.float32)
        n = x.shape[1]
        start = np.broadcast_to(np.asarray(s0, np.float32).reshape(-1, 1), (P, 1))
        end = np.broadcast_to(np.asarray(in1, np.float32).reshape(-1, 1), (P, 1))
        idx = np.arange(n, dtype=np.float32)[None, :]
        mask = (idx >= np.minimum(start, end)) & (idx < np.maximum(start, end))
        mask = np.where(start > end, ~mask, mask)
        body = np.where(mask, x, 0.0) * np.float32(imm2)
        acc = np.asarray(s1, np.float32).reshape(-1, 1) + body.sum(1, keepdims=True)
        return body.reshape(in0.shape), acc.astype(np.float32)

    _mask_idx = ((Idx >= minn(C0, C3)) & (Idx < maxx(C0, C3))) ^ (C0 > C3)
    body = _spill_c3_to_src1(select(_mask_idx, dve_ops.Src0, Zero) * C2)
    spec = Spec(body=body, accum=add, accum_init=C1, reference=_ref)
    shas = {}
    for ver in ("v3", "v4"):
        try:
            shas[ver] = DveOpSpec(name=name, opcode=0, uops=lower(spec, ver=ver),
                                  rd1_en=True).sha(ver)
        except Exception:
            pass
    op = DveOp(name, spec, subdim=False, uops_sha=shas)
    OPS.append(op)
    _SUB_OPCODE_FOR_NAME[name] = _CUSTOM_DVE_ROW_BASE + len(OPS) - 1
    dve_ops.CUSTOM_DVE_SPECS[name] = spec
    return op


def _build():
    """Compile the SPMD Bass graph (once per process)."""
    global _BUILT
    if _BUILT is not None:
        return _BUILT

    import concourse.bacc as bacc
    import concourse.mybir as mybir
    from concourse import tile
    from concourse import dve_ops

    MASK_SUM = _register_mask_sum()
    TMR = dve_ops.TENSOR_MASK_REDUCE

    f32 = mybir.dt.float32
    f32r = mybir.dt.float32r
    bf16 = mybir.dt.bfloat16

    nc = bacc.Bacc("TRN2", target_bir_lowering=False, debug=False,
                   num_devices=NCORES)

    # ---- DRAM I/O -------------------------------------------------------
    d_ftT = nc.dram_tensor("ftT", [D, N], bf16, kind="ExternalInput").ap()
    d_auglhs = nc.dram_tensor("auglhs", [4, RPC], bf16, kind="ExternalInput").ap()
    d_augrhs = nc.dram_tensor("augrhs", [4, N], bf16, kind="ExternalInput").ap()
    # cmean^T k-tiles [128, 64] packed side by side: cmT[:, 64k:64k+64]
    d_cmT = nc.dram_tensor("cmT", [128, 4 * NCLS], bf16, kind="ExternalInput").ap()
    d_gb = nc.dram_tensor("gb", [NCLS, 2], f32r, kind="ExternalInput").ap()
    d_oh = nc.dram_tensor("oh", [RPC, NCLS], f32, kind="ExternalInput").ap()
    d_ohT = nc.dram_tensor("ohT", [NCLS, RPC], f32r, kind="ExternalInput").ap()
    d_rc = nc.dram_tensor("rc", [RPC, 32], f32, kind="ExternalInput").ap()
    d_ones = nc.dram_tensor("onesr", [128, 1], f32r, kind="ExternalInput").ap()
    o_rows = nc.dram_tensor("o_rows", [RPC, 8], f32, kind="ExternalOutput").ap()
    o_cs = nc.dram_tensor("o_cs", [1, N], f32, kind="ExternalOutput").ap()

    with tile.TileContext(nc) as tc:
        with (
            tc.tile_pool(name="const", bufs=1) as cp,
            tc.tile_pool(name="dist", bufs=14) as dp,
            tc.tile_pool(name="scr", bufs=4) as sp,
            tc.tile_pool(name="acc", bufs=2) as ap_,
            tc.tile_pool(name="fin", bufs=1) as fp,
            tc.tile_pool(name="psh", bufs=6, space="PSUM") as psh,
            tc.tile_pool(name="pss", bufs=1, space="PSUM") as pss,
            tc.tile_pool(name="psc", bufs=1, space="PSUM") as psc,
        ):
            # ---- load constants -----------------------------------------
            # ftT: 4 large DMAs (one per k-tile) so the HW DGE fans each out
            # across many queues; small constants ride other sequencers.
            ft = [cp.tile([128, N], bf16, tag=f"ft{k}", name=f"ft{k}") for k in range(4)]
            # chunk A (local cols 0..1023: own rows + window region) lands
            # first so the main loop starts early; B and C follow.
            for k in range(4):
                nc.sync.dma_start(ft[k][:, 0:640], d_ftT[k * 128:(k + 1) * 128, 0:640])
            for k in range(4):
                nc.sync.dma_start(ft[k][:, 640:1280],
                                  d_ftT[k * 128:(k + 1) * 128, 640:1280])
            for k in range(4):
                nc.scalar.dma_start(ft[k][:, 1280:2560],
                                    d_ftT[k * 128:(k + 1) * 128, 1280:2560])
            for k in range(4):
                nc.sync.dma_start(ft[k][:, 2560:N],
                                  d_ftT[k * 128:(k + 1) * 128, 2560:N])
            auglhs = cp.tile([4, RPC], bf16)
            augrhs = cp.tile([4, N], bf16)
            nc.gpsimd.dma_start(auglhs[:], d_auglhs[:])
            nc.gpsimd.dma_start(augrhs[:], d_augrhs[:])
            oh = [cp.tile([128, NCLS], f32, tag=f"oh{k}", name=f"oh{k}") for k in range(4)]
            for k in range(4):
                nc.scalar.dma_start(oh[k][:], d_oh[k * 128:(k + 1) * 128, :])
            ohT = cp.tile([NCLS, RPC], f32r)
            nc.scalar.dma_start(ohT[:], d_ohT[:])
            # per-row constants, one [128, 32] tile per rowblock:
            # [0:8]=mns [8:16]=mne [16:18]=wps [18:20]=wpe [20]=sown
            rc = [cp.tile([128, 32], f32, tag=f"rc{r}", name=f"rc{r}") for r in range(RB)]
            for r in range(RB):
                nc.gpsimd.dma_start(rc[r][:], d_rc[r * 128:(r + 1) * 128, :])
            onesr = cp.tile([128, 1], f32r)
            nc.gpsimd.dma_start(onesr[:], d_ones[:])
            cmTg = cp.tile([128, 4 * NCLS], bf16)
            nc.sync.dma_start(cmTg[:], d_cmT[:])
            cmT = [cmTg[:, k * NCLS:(k + 1) * NCLS] for k in range(4)]
            gb = cp.tile([NCLS, 2], f32r)
            nc.sync.dma_start(gb[:], d_gb[:])
            mns = [rc[r][:, 0:NT] for r in range(RB)]
            mne = [rc[r][:, NT:2 * NT] for r in range(RB)]
            wps = [rc[r][:, 16:17] for r in range(RB)]
            wpe = [rc[r][:, 17:18] for r in range(RB)]
            sown = [rc[r][:, 20:21] for r in range(RB)]
            epst = cp.tile([128, 1], f32)
            nc.vector.memset(epst[:], EPS)

            orows_sb = [fp.tile([128, 8], f32, tag=f"or{r}", name=f"orows{r}") for r in range(RB)]
            for r in range(RB):
                nc.vector.memset(orows_sb[r][:], 0.0)
            cs_sb = fp.tile([1, N], f32)

            # ================= Phase A: class stats ======================
            # cmean / cm2 / uncertainty are label-stat prep computed on the
            # host; the device does the O(N*C*D) FCM matmul + gathers.
            # per-rowblock: FCM, gathers, stat_margin -> orows col 3
            def emit_stats(r):
                own = slice(128 + r * 128, 256 + r * 128)   # local cols of own rows
                fcm_ps = pss.tile([128, NCLS], f32, tag="stat")
                for k in range(4):
                    nc.tensor.matmul(fcm_ps[:], ft[k][:, own], cmT[k],
                                     start=(k == 0), stop=(k == 3))
                g_ps = pss.tile([128, 2], f32, tag="stat")
                nc.tensor.matmul(g_ps[:], ohT[:, r * 128:(r + 1) * 128], gb[:],
                                 start=True, stop=True)
                scr64 = sp.tile([128, NCLS], f32, tag="scr64")
                nc.vector.scalar_tensor_tensor(
                    out=scr64[:], in0=oh[r][:], scalar=1.0, in1=fcm_ps[:],
                    op0=mybir.AluOpType.mult, op1=mybir.AluOpType.mult)
                fcm_g = ap_.tile([128, 1], f32, tag=f"fg{r}")
                nc.vector.tensor_reduce(fcm_g[:], scr64[:], axis=mybir.AxisListType.X,
                                        op=mybir.AluOpType.add)
                cd2 = ap_.tile([128, 1], f32, tag=f"cd{r}")
                # cd2 = fcm_g * (-2) + cm2_g
                nc.vector.scalar_tensor_tensor(
                    out=cd2[:], in0=fcm_g[:], scalar=-2.0, in1=g_ps[:, 1:2],
                    op0=mybir.AluOpType.mult, op1=mybir.AluOpType.add)
                nc.vector.tensor_tensor(cd2[:], cd2[:], sown[r][:],
                                        op=mybir.AluOpType.add)
                nc.vector.tensor_scalar_max(cd2[:], cd2[:], 0.0)
                cd = ap_.tile([128, 1], f32, tag=f"cdq{r}")
                nc.scalar.activation(cd[:], cd2[:], mybir.ActivationFunctionType.Sqrt)
                nc.vector.tensor_tensor(orows_sb[r][:, 3:4], cd[:], g_ps[:, 0:1],
                                        op=mybir.AluOpType.mult)

            # ================= Phase B: distance matrix ==================
            maxh = [[None] * (NT + 1) for _ in range(RB)]
            dists = [[None] * RB for _ in range(NT)]
            wdist = [sp.tile([128, 2 * TT], f32r, tag=f"wd{r}", name=f"wd{r}",
                             bufs=1) for r in range(RB)]

            def emit_colsum(t):
                csum_ps = psc.tile([1, TT], f32, tag="cs", name=f"cs{t}")
                for r in range(RB):
                    nc.tensor.matmul(csum_ps[:], onesr[:], dists[t][r],
                                     start=(r == 0), stop=(r == RB - 1))
                nc.scalar.copy(cs_sb[:, t * TT:(t + 1) * TT], csum_ps[:])

            for t in range(NT):
                hps_t = [psh.tile([128, TT], f32, tag="h", name=f"h{t}_{r}")
                         for r in range(RB)]
                if t < 2:
                    # k-layer order: all rowblocks per k so the PE stream has
                    # 4 matmuls of work per arriving ftT chunk
                    cols = slice(t * TT, (t + 1) * TT)
                    for k in range(4):
                        for r in range(RB):
                            own = slice(128 + r * 128, 256 + r * 128)
                            nc.tensor.matmul(hps_t[r][:], ft[k][:, own],
                                             ft[k][:, cols],
                                             start=(k == 0), stop=False)
                    for r in range(RB):
                        nc.tensor.matmul(hps_t[r][:],
                                         auglhs[:, r * 128:(r + 1) * 128],
                                         augrhs[:, cols], start=False, stop=True)
                for r in range(RB):
                    own = slice(128 + r * 128, 256 + r * 128)
                    cols = slice(t * TT, (t + 1) * TT)
                    h_ps = hps_t[r]
                    if t >= 2:
                        for k in range(4):
                            nc.tensor.matmul(h_ps[:], ft[k][:, own], ft[k][:, cols],
                                             start=(k == 0), stop=False)
                        nc.tensor.matmul(h_ps[:], auglhs[:, r * 128:(r + 1) * 128],
                                         augrhs[:, cols], start=False, stop=True)
                    # dist = sqrt(-2h + eps)   (f32r output); the two window
                    # tiles (t<2) land side by side in wdist[r]
                    if t < WT:
                        dt_ = wdist[r][:, t * TT:(t + 1) * TT]
                    else:
                        dt_ = dp.tile([128, TT], f32r, tag="dist", name="dtile")[:]
                    nc.scalar.activation(dt_, h_ps[:],
                                         mybir.ActivationFunctionType.Sqrt,
                                         bias=epst[:], scale=-2.0)
                    # hneg: exclusion masked max of h, chained over t
                    scr = sp.tile([128, TT], f32, tag="scr")
                    prev = maxh[r][t] if t > 0 else -1e30
                    if t == NT - 1:
                        acc_mh = orows_sb[r][:, 1:2]
                    else:
                        nxt = ap_.tile([128, 1], f32, tag=f"mh{r}")
                        maxh[r][t + 1] = nxt
                        acc_mh = nxt[:]
                    nc.vector._custom_dve(
                        TMR, out=scr[:], in0=h_ps[:], in1=mne[r][:, t:t + 1],
                        s0=mns[r][:, t:t + 1],
                        s1=(prev[:] if t > 0 else prev), imm2=1.0,
                        accum_out=acc_mh)
                    if t == WT - 1:
                        # windowed hpos / sum_pos over ONE 512-wide slice of
                        # wdist at column offset 128r (covers the whole class
                        # window of this rowblock's rows)
                        wsl = wdist[r][:, 128 * r:128 * r + TT].bitcast(f32)
                        scr2 = sp.tile([128, TT], f32, tag="scr")
                        nc.vector._custom_dve(
                            TMR, out=scr2[:], in0=wsl, in1=wpe[r],
                            s0=wps[r], s1=-1e30, imm2=1.0,
                            accum_out=orows_sb[r][:, 0:1])
                        scr3 = sp.tile([128, TT], f32, tag="scr")
                        nc.vector._custom_dve(
                            MASK_SUM, out=scr3[:], in0=wsl, in1=wpe[r],
                            s0=wps[r], s1=0.0, imm2=1.0,
                            accum_out=orows_sb[r][:, 2:3])
                    dists[t][r] = dt_
                # column sums (partial row sums by symmetry), one tile behind
                # so the in-order PE stream never waits on ScalarE's sqrt
                if t > 0:
                    emit_colsum(t - 1)
                if t == 2:
                    for rr in range(RB):
                        emit_stats(rr)
            emit_colsum(NT - 1)

            # ---- outputs ------------------------------------------------
            for r in range(RB):
                nc.sync.dma_start(o_rows[r * 128:(r + 1) * 128, :], orows_sb[r][:])
            nc.sync.dma_start(o_cs[:], cs_sb[:])

    nc.compile()
    _BUILT = nc
    return nc


def _split_bf16(x32, mldt):
    hi = x32.astype(mldt.bfloat16)
    lo = (x32 - hi.astype(np.float32)).astype(mldt.bfloat16)
    return hi, lo


def kernel(feats, labels):
    import sys
    if "/opt/trn_rl_repo" not in sys.path:
        sys.path.insert(0, "/opt/trn_rl_repo")
    import ml_dtypes
    from concourse.bass_utils import run_bass_kernel_spmd

    feats_np = np.asarray(feats, dtype=np.float32)
    labels_np = np.asarray(labels)
    lab_i = labels_np.astype(np.int64)
    assert feats_np.shape == (N, D)

    # ---- host prep: sort by class --------------------------------------
    order = np.argsort(lab_i, kind="stable")
    ls = lab_i[order]
    fs = feats_np[order]
    cnt = np.bincount(ls, minlength=NCLS).astype(np.int64)
    seg_start = np.concatenate([[0], np.cumsum(cnt)[:-1]])
    ws_g = seg_start[ls].astype(np.int64)          # per sorted row: class col start
    we_g = (seg_start[ls] + cnt[ls]).astype(np.int64)

    fb = fs.astype(ml_dtypes.bfloat16)             # bf16 feats, sorted rows
    fb32 = fb.astype(np.float32)
    s_b = (fb32.astype(np.float64) ** 2).sum(1)    # ||bf16 f||^2 (f64)
    sh32 = (-(s_b / 2.0)).astype(np.float32)       # -s/2 in f32
    s_f32 = ((fs.astype(np.float64) ** 2).sum(1)).astype(np.float32)  # exact norms
    hi, lo = _split_bf16(sh32, ml_dtypes)

    ftT_g = np.ascontiguousarray(fb.T)             # [D, N] bf16, global col order

    onehot = np.zeros((N, NCLS), np.float32)
    onehot[np.arange(N), ls] = 1.0


    # class stats (host: label-derived prep at O(N*D))
    cnt_f = np.maximum(cnt, 1).astype(np.float64)
    cmean = np.zeros((NCLS, D), np.float64)
    np.add.at(cmean, ls, fs.astype(np.float64))
    cmean /= cnt_f[:, None]
    cmsq = np.zeros((NCLS, D), np.float64)
    np.add.at(cmsq, ls, fs.astype(np.float64) ** 2)
    cmsq /= cnt_f[:, None]
    cvar = np.maximum(cmsq - cmean ** 2, 0.0)        # per-dim clamp like the ref
    cm2 = (cmean ** 2).sum(1)                        # ||cmean_c||^2
    u = cvar.mean(1)                                 # mean_d cvar
    cmT_grp = np.ascontiguousarray(
        cmean.T.reshape(4, 128, NCLS).transpose(1, 0, 2).reshape(128, 4 * NCLS)
    ).astype(ml_dtypes.bfloat16)
    gb_host = np.stack([u, cm2], axis=1).astype(np.float32)

    in_maps = []
    for c in range(NCORES):
        roll = 512 * c - 128
        colperm = (np.arange(N) + roll) % N        # local j -> global col
        rows = slice(512 * c, 512 * (c + 1))
        lw = ws_g[rows] - roll                     # local window bounds per row
        le = we_g[rows] - roll
        assert lw.min() >= 0 and le.max() <= WT * TT, (lw.min(), le.max())

        rc_a = np.zeros((RPC, 32), np.float32)
        mns_a = rc_a[:, 0:NT]
        mne_a = rc_a[:, NT:2 * NT]
        rc_a[:, 20] = s_f32[rows]
        for t in range(NT):
            a = np.clip(lw - t * TT, 0, TT)
            b = np.clip(le - t * TT, 0, TT)
            inter = b > a
            # exclusion encoding for hneg: (start, end) = (b, a); else include-all
            mns_a[:, t] = np.where(inter, b, 0.0)
            mne_a[:, t] = np.where(inter, a, float(TT))
        # window bounds relative to the rowblock's 512-wide slice at 128*rb
        rb_of = np.arange(RPC) // 128
        rel_s = lw - 128 * rb_of
        rel_e = le - 128 * rb_of
        assert rel_s.min() >= 0 and rel_e.max() <= TT, (rel_s.min(), rel_e.max())
        rc_a[:, 16] = rel_s
        rc_a[:, 17] = rel_e

        augrhs = np.zeros((4, N), ml_dtypes.bfloat16)
        augrhs[0, :] = ml_dtypes.bfloat16(1.0)
        augrhs[1, :] = ml_dtypes.bfloat16(1.0)
        augrhs[2, :] = hi[colperm]
        augrhs[3, :] = lo[colperm]
        auglhs = np.zeros((4, RPC), ml_dtypes.bfloat16)
        auglhs[0, :] = hi[rows]
        auglhs[1, :] = lo[rows]
        auglhs[2, :] = ml_dtypes.bfloat16(1.0)
        auglhs[3, :] = ml_dtypes.bfloat16(1.0)

        in_maps.append({
            "ftT": np.ascontiguousarray(ftT_g[:, colperm]),
            "auglhs": auglhs,
            "augrhs": augrhs,
            "cmT": cmT_grp,
            "gb": gb_host,
            "oh": onehot[rows],
            "ohT": np.ascontiguousarray(onehot[rows].T.astype(np.float32)),
            "rc": rc_a,
            "onesr": np.ones((128, 1), np.float32),
        })

    nc = _build()
    trace = _maybe_enable_trace()
    import tempfile
    tmpdir = tempfile.mkdtemp(prefix="triplet_trace_") if trace else None
    res = run_bass_kernel_spmd(nc, in_maps, core_ids=list(range(NCORES)),
                               trace=bool(trace), tmpdir=tmpdir)
    global LAST_EXEC_NS, LAST_TRACE_DIR
    LAST_EXEC_NS = res.exec_time_ns
    LAST_TRACE_DIR = tmpdir

    # ---- host epilogue (O(N) numpy) ------------------------------------
    hpos = np.concatenate([res.results[c]["o_rows"][:, 0] for c in range(NCORES)])
    maxh = np.concatenate([res.results[c]["o_rows"][:, 1] for c in range(NCORES)])
    possum = np.concatenate([res.results[c]["o_rows"][:, 2] for c in range(NCORES)])
    statm = np.concatenate([res.results[c]["o_rows"][:, 3] for c in range(NCORES)])

    rowsum = np.zeros(N, np.float64)
    for c in range(NCORES):
        roll = 512 * c - 128
        colperm = (np.arange(N) + roll) % N
        part = res.results[c]["o_cs"][0].astype(np.float64)
        np.add.at(rowsum, colperm, part)

    hneg = np.sqrt(np.maximum(-2.0 * maxh.astype(np.float64) + EPS, 0.0))
    diag_dist = np.sqrt(EPS)
    pos_cnt = (cnt[ls] - 1).astype(np.float64)
    neg_cnt = (N - cnt[ls]).astype(np.float64)
    mean_pos = (possum.astype(np.float64) - diag_dist) / np.maximum(pos_cnt, 1.0)
    sum_neg = rowsum - possum.astype(np.float64)
    mean_neg = sum_neg / np.maximum(neg_cnt, 1.0)
    final_margin = (BASE_MARGIN + ADAPTIVE_WEIGHT * (mean_neg - mean_pos)
                    + STAT_WEIGHT * statm.astype(np.float64))
    per_sample = np.maximum(hpos.astype(np.float64) - hneg + final_margin, 0.0)
    valid = (pos_cnt > 0) & (neg_cnt > 0)
    n_valid = valid.sum()
    loss = per_sample[valid].sum() / max(n_valid, 1) if n_valid > 0 else 0.0
    return np.array(loss, dtype=np.float32)


if __name__ == "__main__":
    import jax
    key = jax.random.key(0)
    k1, k2 = jax.random.split(key)
    feats = np.asarray(jax.random.normal(k1, (N, D), dtype=np.float32))
    labels = np.asarray(jax.random.randint(k2, (N,), 0, NCLS, dtype=np.int32))
    out = kernel(feats=feats, labels=labels)
    print("kernel loss:", out)
